# revision 8
# baseline (speedup 1.0000x reference)
"""Trainium2 Bass kernel for nn_DynaResidualBlockC (hyper-network dynamic
residual block).

Strategy (8 NeuronCores, data-parallel over batch; core c owns samples
2c, 2c+1):

  * The ACT (scalar) engine is the hard floor: 4 full-width SIN passes
    (2 waves x cos/sin) = 4 x 128 x 36864 elements per core at 1 elem/
    cycle/lane @ 1.2 GHz ~= 123 us.  Everything else is organized so ACT
    never waits:
      - activations run at FD=1024 straight from PSUM (amortizes the
        ~172-cycle per-instruction init),
      - an explicit same-engine ordering chain keeps ACT in
        w1c,w1s,w2c,w2s order so single-buffered PSUM never stalls it,
      - a deep (D=19 super-tile) wave-1 run-ahead decouples ACT from the
        weight-delivery latencies below.
  * Collectives on this platform complete no earlier than ~65 us wall
    clock (launch skew / CC init), so the ACT-critical weights avoid
    them entirely: the k_in and k_mid hypernet blocks (12.6 MB bf16) are
    REPLICATED to every core, which computes its own 2 samples' kernels
    directly (lat2 lhsT).  Hypernet biases likewise (tiny).  Only the
    out-stage weights (k_out, k_short), whose consumption naturally lags
    by D+2 super-tiles, go through a sharded hypernet + one AllToAll.
  * Main loop, per 1024-column super-tile:
        psum_in  = W_in.T @ x2                        (PE, bf16)
        w1c/w1s  = sin(psum_in + b_in (+pi/2))        (ACT, fused bias)
        psum_mid = W_mid_c.T @ w1c + W_mid_s.T @ w1s
        w2c/w2s  = sin(psum_mid + b_mid (+pi/2))
        psum_out = W_out_c.T @ w2c + W_out_s.T @ w2s + W_short.T @ x2
        y        = psum_out + (b_out + b_short)       (DVE, bf16 out)
    PSUM: ps_in x1 (+ ramp alternate in the ps_out slot) + ps_mid x2 +
    ps_out x1 = 8 banks exactly.
  * k_mid hypernet chunks are dripped through the main-loop blocks
    (1 per block) so their DMA and compute overlap the wave-1 ramp.
  * y leaves the device as bf16 and is cast to fp32 on the host.
"""
import ml_dtypes
import numpy as np

import concourse.bass as bass
import concourse.bacc as bacc
import concourse.mybir as mybir
from concourse import tile
from concourse.bass_utils import run_bass_kernel_spmd

# ---------------------------------------------------------------- constants
B, FIN, FOUT, FH, H2 = 16, 64, 64, 128, 64
LAT = 512
HH = WW = 192
SP = HH * WW                      # 36864 spatial positions
NCORES = 8
S = 1024                          # spatial columns per super-tile
NT = SP // S                      # 36
D = 19                            # wave-1 run-ahead depth (super-tiles)
KIM = 12288                       # replicated hypernet cols (k_in + k_mid)
KOS = 1536                        # sharded hypernet cols per core
NKMID = 16                        # 512-col k_mid chunks
PI_2 = float(np.pi / 2)

F32 = mybir.dt.float32
BF16 = mybir.dt.bfloat16
NP_BF16 = ml_dtypes.bfloat16


def _build_indices():
    """Original Wk-row index for each device column, plus row scales."""
    idx_kim = np.zeros(KIM, np.int64)
    c = np.arange(4096)
    idx_kim[c] = (c % 64) * 64 + c // 64                      # k_in.T
    idx_kim[4096 + c] = 4096 + (c % 64) * 128 + c // 64       # mid cos
    idx_kim[8192 + c] = 4096 + (c % 64) * 128 + 64 + c // 64  # mid sin
    idx_os = np.zeros((NCORES, KOS), np.int64)
    for s in range(NCORES):
        c = np.arange(1024)
        i_l, o = c // 64, c % 64
        if s < 4:
            idx_os[s, c] = 12288 + o * 128 + 16 * s + i_l     # out cos
        else:
            idx_os[s, c] = 12288 + o * 128 + 64 + 16 * (s - 4) + i_l
        cl = np.arange(512)
        i_l, o = cl // 64, cl % 64
        idx_os[s, 1024 + cl] = 20480 + o * 64 + 8 * s + i_l   # short
    scale = np.ones(24832, np.float32)
    scale[:12288] = 1.0 / np.sqrt(128.0)       # k_in, k_mid
    scale[12288:24576] = 1.0 / 8.0             # k_out, k_short
    return idx_kim, idx_os, scale


def _chain(prev, cur):
    if prev is not None:
        tile.add_dep_helper(cur.ins, prev.ins, sync=False,
                            reason="act-order")
    return cur


def _build_nc():
    nc = bacc.Bacc(
        "TRN2",
        target_bir_lowering=False,
        debug=False,
        num_devices=NCORES,
    )
    x_d = nc.dram_tensor("x", [128, SP], BF16, kind="ExternalInput")
    latT_d = nc.dram_tensor("latT", [LAT, B], BF16, kind="ExternalInput")
    lat2_d = nc.dram_tensor("lat2", [LAT, 2], BF16, kind="ExternalInput")
    kimT_d = nc.dram_tensor("kimT", [LAT, KIM], BF16, kind="ExternalInput")
    bkim_d = nc.dram_tensor("bkim", [1, KIM], BF16, kind="ExternalInput")
    wosT_d = nc.dram_tensor("wosT", [LAT, KOS], BF16, kind="ExternalInput")
    bkos_d = nc.dram_tensor("bkos", [1, KOS], BF16, kind="ExternalInput")
    bT_d = nc.dram_tensor("bT", [LAT, 256], BF16, kind="ExternalInput")
    bk2_d = nc.dram_tensor("bk2", [1, 256], BF16, kind="ExternalInput")
    ones2_d = nc.dram_tensor("ones2", [1, 2], BF16, kind="ExternalInput")
    ones16_d = nc.dram_tensor("ones16", [1, B], BF16, kind="ExternalInput")
    zeros_d = nc.dram_tensor("zeros", [16, 16], BF16, kind="ExternalInput")
    y_d = nc.dram_tensor("y", [128, SP], BF16, kind="ExternalOutput")

    SIN = mybir.ActivationFunctionType.Sin

    with tile.TileContext(nc) as tc:
        with (
            tc.tile_pool(name="const", bufs=1) as cpool,
            tc.tile_pool(name="wts", bufs=1) as w_pool,
            tc.tile_pool(name="kim", bufs=3) as kim_pool,
            tc.tile_pool(name="bkp", bufs=3) as bk_pool,
            tc.tile_pool(name="stg", bufs=3) as stg_pool,
            tc.tile_pool(name="dram", bufs=1, space="DRAM") as dram_pool,
            tc.tile_pool(name="psA", bufs=1, space=bass.MemorySpace.PSUM) as psA,
            tc.tile_pool(name="psB", bufs=2, space=bass.MemorySpace.PSUM) as psB,
            tc.tile_pool(name="psC", bufs=1, space=bass.MemorySpace.PSUM) as psC,
            tc.tile_pool(name="xin", bufs=3) as x_pool,
            tc.tile_pool(name="xin2", bufs=3) as x2_pool,
            tc.tile_pool(name="w1", bufs=D + 2) as w1_pool,
            tc.tile_pool(name="w2", bufs=6) as w2_pool,
            tc.tile_pool(name="outs", bufs=3) as out_pool,
        ):
            # ---- dummy warm-up collective --------------------------------
            ccd_in = dram_pool.tile([B, 16], BF16, name="ccd_in")
            ccd_out = dram_pool.tile([B, 16], BF16, name="ccd_out")
            nc.gpsimd.dma_start(ccd_in[:], zeros_d[:])
            nc.gpsimd.collective_compute(
                "AllToAll",
                mybir.AluOpType.bypass,
                replica_groups=[list(range(NCORES))],
                ins=[ccd_in.opt()],
                outs=[ccd_out.opt()],
            )

            # ---- small consts on the gpsimd queue ------------------------
            lat_tiles, lat2_tiles, bT_tiles = [], [], []
            for q in range(4):
                lt = cpool.tile([128, B], BF16, name=f"lat{q}", tag=f"lat{q}")
                nc.gpsimd.dma_start(lt[:], latT_d[128 * q:128 * (q + 1), :])
                lat_tiles.append(lt)
                l2 = cpool.tile([128, 2], BF16, name=f"lat2_{q}",
                                tag=f"lat2_{q}")
                nc.gpsimd.dma_start(l2[:], lat2_d[128 * q:128 * (q + 1), :])
                lat2_tiles.append(l2)
                bt = cpool.tile([128, 256], BF16, name=f"bT{q}", tag=f"bT{q}")
                nc.gpsimd.dma_start(bt[:], bT_d[128 * q:128 * (q + 1), :])
                bT_tiles.append(bt)
            ones2 = cpool.tile([1, 2], BF16, name="ones2")
            nc.gpsimd.dma_start(ones2[:], ones2_d[0:1, 0:2])
            ones16 = cpool.tile([1, B], BF16, name="ones16")
            nc.gpsimd.dma_start(ones16[:], ones16_d[:])
            bk2 = cpool.tile([1, 256], BF16, name="bk2")
            nc.gpsimd.dma_start(bk2[:], bk2_d[:])
            bkos = cpool.tile([1, KOS], BF16, name="bkos")
            nc.gpsimd.dma_start(bkos[:], bkos_d[:])

            # pre-trigger the trig ACT table load
            zscratch = cpool.tile([1, 2], F32, name="zscratch")
            nc.scalar.activation(zscratch[:], ones2[:], SIN, bias=0.0)

            # ---- per-core bias hypernet (tiny, replicated) ---------------
            ps_b = psB.tile([2, 256], F32, name="ps_b", tag="psB")
            for q in range(4):
                nc.tensor.matmul(ps_b[:], lat2_tiles[q][:], bT_tiles[q][:],
                                 start=(q == 0), stop=False)
            nc.tensor.matmul(ps_b[:], ones2[:], bk2[:], start=False,
                             stop=True)
            ksb = cpool.tile([2, 256], BF16, name="ksb")
            nc.vector.tensor_copy(ksb[:], ps_b[:])

            vin = cpool.tile([128, 1], F32, name="vin")
            vmid = cpool.tile([128, 1], F32, name="vmid")
            vout = cpool.tile([128, 1], F32, name="vout")
            vsh = cpool.tile([128, 1], F32, name="vsh")
            cvin = cpool.tile([128, 1], F32, name="cvin")
            cvmid = cpool.tile([128, 1], F32, name="cvmid")
            obias = cpool.tile([128, 1], F32, name="obias")
            for smp in (0, 1):
                for q, dest in enumerate([vin, vmid, vout, vsh]):
                    nc.gpsimd.dma_start(
                        dest[64 * smp:64 * smp + 64, 0:1],
                        ksb[smp:smp + 1, 64 * q:64 * q + 64],
                    )
            nc.vector.tensor_scalar_add(cvin[:], vin[:], PI_2)
            nc.vector.tensor_scalar_add(cvmid[:], vmid[:], PI_2)
            nc.vector.tensor_add(obias[:], vout[:], vsh[:])

            # ---- sharded out/short hypernet + single AllToAll ------------
            wos_tiles = []
            for q in range(4):
                wt = cpool.tile([128, KOS], BF16, name=f"wos{q}",
                                tag=f"wos{q}")
                nc.sync.dma_start(wt[:], wosT_d[128 * q:128 * (q + 1), :])
                wos_tiles.append(wt)
            cc_in = dram_pool.tile([B, KOS], BF16, name="cc_in")
            cc_out = dram_pool.tile([B, KOS], BF16, name="cc_out")
            ks_os = cpool.tile([B, KOS], BF16, name="ks_os")
            for n0 in range(0, KOS, 512):
                ps = psB.tile([B, 512], F32, name="osps", tag="psB")
                for q in range(4):
                    nc.tensor.matmul(ps[:], lat_tiles[q][:],
                                     wos_tiles[q][:, n0:n0 + 512],
                                     start=(q == 0), stop=False)
                nc.tensor.matmul(ps[:], ones16[:], bkos[:, n0:n0 + 512],
                                 start=False, stop=True)
                nc.vector.tensor_copy(ks_os[:, n0:n0 + 512], ps[:])
                nc.gpsimd.dma_start(cc_in[:, n0:n0 + 512],
                                    ks_os[:, n0:n0 + 512])
            nc.gpsimd.collective_compute(
                "AllToAll",
                mybir.AluOpType.bypass,
                replica_groups=[list(range(NCORES))],
                ins=[cc_in.opt()],
                outs=[cc_out.opt()],
            )

            # ---- replicated-hypernet piece machinery ---------------------
            # kim piece p = kimT cols [1024p, 1024(p+1)): 4 SBUF band tiles.
            # k_in = pieces 0..3 (scalar queue), k_mid = 4..11 (sync queue).
            kim_pieces, bkim_pieces = {}, {}

            def load_kim_piece(p, queue):
                tiles = []
                for q in range(4):
                    kt = kim_pool.tile([128, 1024], BF16, name="kimp",
                                       tag=f"kimp{q}")
                    queue.dma_start(
                        kt[:], kimT_d[128 * q:128 * (q + 1),
                                      1024 * p:1024 * (p + 1)])
                    tiles.append(kt)
                kim_pieces[p] = tiles
                bt = bk_pool.tile([1, 1024], BF16, name="bkimp", tag="bkimp")
                nc.gpsimd.dma_start(
                    bt[:], bkim_d[0:1, 1024 * p:1024 * (p + 1)])
                bkim_pieces[p] = bt

            def hyper_chunk(n, dest_sb, dest_col):
                """512-col own-sample hypernet chunk n -> bf16 SBUF dest."""
                p, j = n // 2, n % 2
                kts, bt = kim_pieces[p], bkim_pieces[p]
                ps = psB.tile([2, 512], F32, name="kinps", tag="psB")
                for q in range(4):
                    nc.tensor.matmul(
                        ps[:], lat2_tiles[q][:],
                        kts[q][:, 512 * j:512 * (j + 1)],
                        start=(q == 0), stop=False)
                nc.tensor.matmul(ps[:], ones2[:],
                                 bt[:, 512 * j:512 * (j + 1)],
                                 start=False, stop=True)
                nc.vector.tensor_copy(dest_sb[:, dest_col:dest_col + 512],
                                      ps[:])
                if j == 1:
                    del kim_pieces[p], bkim_pieces[p]

            # ---- k_in hypernet (prologue) --------------------------------
            ks_in = cpool.tile([2, 4096], BF16, name="ks_in")
            ksd_mid = dram_pool.tile([2, 8192], BF16, name="ksd_mid")
            for p in range(4):
                load_kim_piece(p, nc.scalar)
                hyper_chunk(2 * p, ks_in, 1024 * p)
                hyper_chunk(2 * p + 1, ks_in, 1024 * p + 512)

            # ---- weight tiles --------------------------------------------
            W_in = w_pool.tile([128, 128], BF16, name="W_in")
            W_mid_c = w_pool.tile([128, 128], BF16, name="W_mid_c")
            W_mid_s = w_pool.tile([128, 128], BF16, name="W_mid_s")
            W_out_c = w_pool.tile([128, 128], BF16, name="W_out_c")
            W_out_s = w_pool.tile([128, 128], BF16, name="W_out_s")
            W_short = w_pool.tile([128, 128], BF16, name="W_short")
            for Wt in (W_in, W_mid_c, W_mid_s, W_out_c, W_out_s, W_short):
                nc.gpsimd.memset(Wt[0:64, 64:128], 0.0)
                nc.gpsimd.memset(Wt[64:128, 0:64], 0.0)
            for smp in (0, 1):
                dg = np.s_[64 * smp:64 * smp + 64, 64 * smp:64 * smp + 64]
                nc.gpsimd.dma_start(W_in[dg], ks_in[smp:smp + 1, :])

            # ---- main loop -----------------------------------------------
            xts, x2ts, w1s_, w2s_ = {}, {}, {}, {}
            ps_ins, ps_mids = {}, {}
            prev_act = None
            for t in range(NT + D + 3):
                u = t - 1 - D          # mid/w2 tile
                v = t - 2 - D          # out tile

                if 0 <= u < NT:
                    w1cs = w1s_.pop(u)
                    ps_mid = psB.tile([128, S], F32, name="ps_mid",
                                      tag="psB")
                    for h in range(2):
                        sl = np.s_[:, 512 * h:512 * (h + 1)]
                        nc.tensor.matmul(ps_mid[sl], W_mid_c[:],
                                         w1cs[:, 0:S][sl], start=True,
                                         stop=False)
                        nc.tensor.matmul(ps_mid[sl], W_mid_s[:],
                                         w1cs[:, S:2 * S][sl], start=False,
                                         stop=True)
                    ps_mids[u] = ps_mid

                if v == 0:
                    # out/short weight assembly, emitted late so the A2A
                    # wait cannot head-block earlier gpsimd-queue work.
                    for smp in (0, 1):
                        dg = np.s_[64 * smp:64 * smp + 64,
                                   64 * smp:64 * smp + 64]
                        nc.gpsimd.dma_start(W_out_c[dg],
                                            cc_out[smp:8:2, 0:1024])
                        nc.gpsimd.dma_start(W_out_s[dg],
                                            cc_out[8 + smp:16:2, 0:1024])
                        nc.gpsimd.dma_start(W_short[dg],
                                            cc_out[smp:16:2, 1024:1536])

                if 0 <= v < NT:
                    w2cs = w2s_.pop(v)
                    xt_v = x2ts.pop(v)
                    ps_out = psC.tile([128, S], F32, name="ps_out",
                                      tag="psC")
                    for h in range(2):
                        sl = np.s_[:, 512 * h:512 * (h + 1)]
                        nc.tensor.matmul(ps_out[sl], W_out_c[:],
                                         w2cs[:, 0:S][sl], start=True,
                                         stop=False)
                        nc.tensor.matmul(ps_out[sl], W_out_s[:],
                                         w2cs[:, S:2 * S][sl], start=False,
                                         stop=False)
                        nc.tensor.matmul(ps_out[sl], W_short[:], xt_v[sl],
                                         start=False, stop=True)

                if t < NT:
                    xt = x_pool.tile([128, S], BF16, name="xt", tag="xt")
                    nc.sync.dma_start(xt[:], x_d[:, S * t:S * (t + 1)])
                    xts[t] = xt
                    # k_mid piece prefetch, 4 blocks ahead of its use
                    if t % 2 == 0 and 4 + t // 2 <= 11:
                        load_kim_piece(4 + t // 2, nc.sync)
                    pool = psC if (t % 2 == 1 and t < D + 1) else psA
                    ps_in = pool.tile([128, S], F32, name="ps_in",
                                      tag="psC" if pool is psC else "psA")
                    for h in range(2):
                        sl = np.s_[:, 512 * h:512 * (h + 1)]
                        nc.tensor.matmul(ps_in[sl], W_in[:], xt[sl],
                                         start=True, stop=True)
                    ps_ins[t] = ps_in

                if 0 <= t - 1 < NT:
                    w = t - 1
                    w1cs = w1_pool.tile([128, 2 * S], BF16, name="w1",
                                        tag="w1")
                    ps_in_w = ps_ins.pop(w)
                    a = nc.scalar.activation(w1cs[:, 0:S], ps_in_w[:], SIN,
                                             bias=cvin[:, 0:1])
                    prev_act = _chain(prev_act, a)
                    a = nc.scalar.activation(w1cs[:, S:2 * S], ps_in_w[:],
                                             SIN, bias=vin[:, 0:1])
                    prev_act = _chain(prev_act, a)
                    w1s_[w] = w1cs

                if 0 <= u < NT:
                    w2cs = w2_pool.tile([128, 2 * S], BF16, name="w2",
                                        tag="w2")
                    ps_mid_u = ps_mids.pop(u)
                    a = nc.scalar.activation(w2cs[:, 0:S], ps_mid_u[:], SIN,
                                             bias=cvmid[:, 0:1])
                    prev_act = _chain(prev_act, a)
                    a = nc.scalar.activation(w2cs[:, S:2 * S], ps_mid_u[:],
                                             SIN, bias=vmid[:, 0:1])
                    prev_act = _chain(prev_act, a)
                    w2s_[u] = w2cs

                # k_mid hypernet drip: one 512-col chunk per block
                n = t - 4
                if 0 <= n < NKMID:
                    stg = stg_pool.tile([2, 512], BF16, name="stg",
                                        tag="stg")
                    hyper_chunk(8 + n, stg, 0)
                    nc.gpsimd.dma_start(ksd_mid[:, 512 * n:512 * (n + 1)],
                                        stg[:])
                    if n == NKMID - 1:
                        for smp in (0, 1):
                            dg = np.s_[64 * smp:64 * smp + 64,
                                       64 * smp:64 * smp + 64]
                            nc.gpsimd.dma_start(
                                W_mid_c[dg], ksd_mid[smp:smp + 1, 0:4096])
                            nc.gpsimd.dma_start(
                                W_mid_s[dg],
                                ksd_mid[smp:smp + 1, 4096:8192])

                if 0 <= v < NT:
                    ot = out_pool.tile([128, S], BF16, name="ot", tag="ot")
                    nc.vector.tensor_scalar_add(ot[:], ps_out[:],
                                                obias[:, 0:1])
                    nc.sync.dma_start(y_d[:, S * v:S * (v + 1)], ot[:])

                # x re-fetch for the out stage (one block ahead)
                w = t - 1 - D
                if 0 <= w < NT:
                    xt2 = x2_pool.tile([128, S], BF16, name="xt2", tag="xt2")
                    nc.sync.dma_start(xt2[:], x_d[:, S * w:S * (w + 1)])
                    x2ts[w] = xt2

    nc.compile()
    return nc


_NC_CACHE = None


def _get_nc():
    global _NC_CACHE
    if _NC_CACHE is None:
        _NC_CACHE = _build_nc()
    return _NC_CACHE


def kernel(x, lat, Wk, bk, **run_kwargs):
    x = np.asarray(x, dtype=np.float32)
    lat = np.asarray(lat, dtype=np.float32)
    Wk = np.asarray(Wk, dtype=np.float32)
    bk = np.asarray(bk, dtype=np.float32)

    idx_kim, idx_os, scale = _build_indices()
    Wk_s = Wk * scale[:, None]
    bk_s = bk * scale
    latT_b = np.ascontiguousarray(lat.T.astype(NP_BF16))
    x_b = x.reshape(B, FIN * SP).astype(NP_BF16)
    kimT_b = np.ascontiguousarray(Wk_s[idx_kim].T.astype(NP_BF16))
    bkim_b = np.ascontiguousarray(bk_s[idx_kim].reshape(1, KIM)
                                  .astype(NP_BF16))
    bT_b = np.ascontiguousarray(Wk[24576:24832].T.astype(NP_BF16))
    bk2_b = np.ascontiguousarray(bk[24576:24832].reshape(1, 256)
                                 .astype(NP_BF16))

    in_maps = []
    for c in range(NCORES):
        in_maps.append({
            "x": np.ascontiguousarray(
                x_b[2 * c:2 * c + 2].reshape(128, SP)),
            "latT": latT_b,
            "lat2": np.ascontiguousarray(latT_b[:, 2 * c:2 * c + 2]),
            "kimT": kimT_b,
            "bkim": bkim_b,
            "wosT": np.ascontiguousarray(Wk_s[idx_os[c]].T.astype(NP_BF16)),
            "bkos": np.ascontiguousarray(bk_s[idx_os[c]].reshape(1, KOS)
                                         .astype(NP_BF16)),
            "bT": bT_b,
            "bk2": bk2_b,
            "ones2": np.ones((1, 2), NP_BF16),
            "ones16": np.ones((1, B), NP_BF16),
            "zeros": np.zeros((16, 16), NP_BF16),
        })

    nc = _get_nc()
    res = run_bass_kernel_spmd(nc, in_maps, core_ids=list(range(NCORES)),
                               **run_kwargs)
    y = np.empty((B, FOUT, HH, WW), np.float32)
    for c in range(NCORES):
        y[2 * c:2 * c + 2] = (res.results[c]["y"].astype(np.float32)
                              .reshape(2, FOUT, HH, WW))
    if run_kwargs:
        kernel.last_results = res
    return y


# revision 12
# speedup vs baseline: 1.1928x; 1.1928x over previous
"""Trainium2 Bass kernel for nn_DynaResidualBlockC (hyper-network dynamic
residual block).

Strategy (8 NeuronCores, data-parallel over batch; core c owns samples
2c, 2c+1):

  * The ACT (scalar) engine is the hard floor: 4 full-width SIN passes
    (2 waves x cos/sin) = 4 x 128 x 36864 elements per core at 1 elem/
    cycle/lane @ 1.2 GHz ~= 123 us.  Everything else is organized so ACT
    never waits:
      - activations run at FD=1024 straight from PSUM (amortizes the
        ~172-cycle per-instruction init),
      - an explicit same-engine ordering chain keeps ACT in
        w1c,w1s,w2c,w2s order so single-buffered PSUM never stalls it,
      - a deep (D=19 super-tile) wave-1 run-ahead decouples ACT from the
        weight-delivery latencies below.
  * Collectives on this platform complete no earlier than ~65 us wall
    clock (launch skew / CC init), so the ACT-critical weights avoid
    them entirely: the k_in and k_mid hypernet blocks (12.6 MB bf16) are
    REPLICATED to every core, which computes its own 2 samples' kernels
    directly (lat2 lhsT).  Hypernet biases likewise (tiny).  Only the
    out-stage weights (k_out, k_short), whose consumption naturally lags
    by D+2 super-tiles, go through a sharded hypernet + one AllToAll.
  * Main loop, per 1024-column super-tile:
        psum_in  = W_in.T @ x2                        (PE, bf16)
        w1c/w1s  = sin(psum_in + b_in (+pi/2))        (ACT, fused bias)
        psum_mid = W_mid_c.T @ w1c + W_mid_s.T @ w1s
        w2c/w2s  = sin(psum_mid + b_mid (+pi/2))
        psum_out = W_out_c.T @ w2c + W_out_s.T @ w2s + W_short.T @ x2
        y        = psum_out + (b_out + b_short)       (DVE, bf16 out)
    PSUM: ps_in x1 (+ ramp alternate in the ps_out slot) + ps_mid x2 +
    ps_out x1 = 8 banks exactly.
  * k_mid hypernet chunks are dripped through the main-loop blocks
    (1 per block) so their DMA and compute overlap the wave-1 ramp.
  * y leaves the device as bf16 and is cast to fp32 on the host.
"""
import ml_dtypes
import numpy as np

import concourse.bass as bass
import concourse.bacc as bacc
import concourse.mybir as mybir
from concourse import tile
from concourse.bass_utils import run_bass_kernel_spmd

# ---------------------------------------------------------------- constants
B, FIN, FOUT, FH, H2 = 16, 64, 64, 128, 64
LAT = 512
HH = WW = 192
SP = HH * WW                      # 36864 spatial positions
NCORES = 8
S = 1024                          # spatial columns per super-tile
NT = SP // S                      # 36
D = 19                            # wave-1 run-ahead depth (super-tiles)
KIM = 12288                       # replicated hypernet cols (k_in + k_mid)
KOS = 1536                        # sharded hypernet cols per core
NKMID = 16                        # 512-col k_mid chunks
PI_2 = float(np.pi / 2)

F32 = mybir.dt.float32
BF16 = mybir.dt.bfloat16
NP_BF16 = ml_dtypes.bfloat16


def _build_indices():
    """Original Wk-row index for each device column, plus row scales."""
    idx_kim = np.zeros(KIM, np.int64)
    c = np.arange(4096)
    idx_kim[c] = (c % 64) * 64 + c // 64                      # k_in.T
    idx_kim[4096 + c] = 4096 + (c % 64) * 128 + c // 64       # mid cos
    idx_kim[8192 + c] = 4096 + (c % 64) * 128 + 64 + c // 64  # mid sin
    idx_os = np.zeros((NCORES, KOS), np.int64)
    for s in range(NCORES):
        c = np.arange(1024)
        i_l, o = c // 64, c % 64
        if s < 4:
            idx_os[s, c] = 12288 + o * 128 + 16 * s + i_l     # out cos
        else:
            idx_os[s, c] = 12288 + o * 128 + 64 + 16 * (s - 4) + i_l
        cl = np.arange(512)
        i_l, o = cl // 64, cl % 64
        idx_os[s, 1024 + cl] = 20480 + o * 64 + 8 * s + i_l   # short
    scale = np.ones(24832, np.float32)
    scale[:12288] = 1.0 / np.sqrt(128.0)       # k_in, k_mid
    scale[12288:24576] = 1.0 / 8.0             # k_out, k_short
    return idx_kim, idx_os, scale


def _chain(prev, cur):
    if prev is not None:
        tile.add_dep_helper(cur.ins, prev.ins, sync=False,
                            reason="act-order")
    return cur


def _build_nc():
    nc = bacc.Bacc(
        "TRN2",
        target_bir_lowering=False,
        debug=False,
        num_devices=NCORES,
    )
    x_d = nc.dram_tensor("x", [128, SP], BF16, kind="ExternalInput")
    latT_d = nc.dram_tensor("latT", [LAT, B], BF16, kind="ExternalInput")
    lat2_d = nc.dram_tensor("lat2", [LAT, 2], BF16, kind="ExternalInput")
    kimT_d = nc.dram_tensor("kimT", [LAT, KIM], BF16, kind="ExternalInput")
    bkim_d = nc.dram_tensor("bkim", [1, KIM], BF16, kind="ExternalInput")
    wosT_d = nc.dram_tensor("wosT", [LAT, KOS], BF16, kind="ExternalInput")
    bkos_d = nc.dram_tensor("bkos", [1, KOS], BF16, kind="ExternalInput")
    bT_d = nc.dram_tensor("bT", [LAT, 256], BF16, kind="ExternalInput")
    bk2_d = nc.dram_tensor("bk2", [1, 256], BF16, kind="ExternalInput")
    ones2_d = nc.dram_tensor("ones2", [1, 2], BF16, kind="ExternalInput")
    ones16_d = nc.dram_tensor("ones16", [1, B], BF16, kind="ExternalInput")
    zeros_d = nc.dram_tensor("zeros", [16, 16], BF16, kind="ExternalInput")
    y_d = nc.dram_tensor("y", [128, SP], BF16, kind="ExternalOutput")

    SIN = mybir.ActivationFunctionType.Sin

    # gpsimd-queue ordering chain: the Tile scheduler's priority heap can
    # hoist collective-gated DMAs ahead of earlier-emitted ones on the same
    # queue, head-of-line blocking them; pin emission order explicitly.
    gq_prev = [None]

    with tile.TileContext(nc) as tc:
        with (
            tc.tile_pool(name="const", bufs=1) as cpool,
            tc.tile_pool(name="wts", bufs=1) as w_pool,
            tc.tile_pool(name="kim", bufs=3) as kim_pool,
            tc.tile_pool(name="bkp", bufs=3) as bk_pool,
            tc.tile_pool(name="stg", bufs=3) as stg_pool,
            tc.tile_pool(name="dram", bufs=1, space="DRAM") as dram_pool,
            tc.tile_pool(name="psA", bufs=1, space=bass.MemorySpace.PSUM) as psA,
            tc.tile_pool(name="psB", bufs=2, space=bass.MemorySpace.PSUM) as psB,
            tc.tile_pool(name="psC", bufs=2, space=bass.MemorySpace.PSUM) as psC,
            tc.tile_pool(name="xin", bufs=3) as x_pool,
            tc.tile_pool(name="xin2", bufs=3) as x2_pool,
            tc.tile_pool(name="w1", bufs=D + 2) as w1_pool,
            tc.tile_pool(name="w2", bufs=7) as w2_pool,
            tc.tile_pool(name="outs", bufs=2) as out_pool,
        ):
            def gdma(dst, src):
                h = nc.gpsimd.dma_start(dst, src)
                if gq_prev[0] is not None:
                    tile.add_dep_helper(h.ins, gq_prev[0].ins, sync=False,
                                        reason="gq-order")
                gq_prev[0] = h
                return h
            # ---- dummy warm-up collective --------------------------------
            ccd_in = dram_pool.tile([B, 16], BF16, name="ccd_in")
            ccd_out = dram_pool.tile([B, 16], BF16, name="ccd_out")
            gdma(ccd_in[:], zeros_d[:])
            nc.gpsimd.collective_compute(
                "AllToAll",
                mybir.AluOpType.bypass,
                replica_groups=[list(range(NCORES))],
                ins=[ccd_in.opt()],
                outs=[ccd_out.opt()],
            )

            # ---- small consts on the gpsimd queue ------------------------
            lat_tiles, lat2_tiles, bT_tiles = [], [], []
            for q in range(4):
                lt = cpool.tile([128, B], BF16, name=f"lat{q}", tag=f"lat{q}")
                gdma(lt[:], latT_d[128 * q:128 * (q + 1), :])
                lat_tiles.append(lt)
                l2 = cpool.tile([128, 2], BF16, name=f"lat2_{q}",
                                tag=f"lat2_{q}")
                gdma(l2[:], lat2_d[128 * q:128 * (q + 1), :])
                lat2_tiles.append(l2)
                bt = cpool.tile([128, 256], BF16, name=f"bT{q}", tag=f"bT{q}")
                gdma(bt[:], bT_d[128 * q:128 * (q + 1), :])
                bT_tiles.append(bt)
            ones2 = cpool.tile([1, 2], BF16, name="ones2")
            gdma(ones2[:], ones2_d[0:1, 0:2])
            ones16 = cpool.tile([1, B], BF16, name="ones16")
            gdma(ones16[:], ones16_d[:])
            bk2 = cpool.tile([1, 256], BF16, name="bk2")
            gdma(bk2[:], bk2_d[:])
            bkos = cpool.tile([1, KOS], BF16, name="bkos")
            gdma(bkos[:], bkos_d[:])

            # pre-trigger the trig ACT table load
            zscratch = cpool.tile([1, 2], F32, name="zscratch")
            nc.scalar.activation(zscratch[:], ones2[:], SIN, bias=0.0)

            # ---- per-core bias hypernet (tiny, replicated) ---------------
            ps_b = psC.tile([2, 256], F32, name="ps_b", tag="psC")
            for q in range(4):
                nc.tensor.matmul(ps_b[:], lat2_tiles[q][:], bT_tiles[q][:],
                                 start=(q == 0), stop=False)
            nc.tensor.matmul(ps_b[:], ones2[:], bk2[:], start=False,
                             stop=True)
            ksb = cpool.tile([2, 256], BF16, name="ksb")
            nc.vector.tensor_copy(ksb[:], ps_b[:])

            vin = cpool.tile([128, 1], F32, name="vin")
            vmid = cpool.tile([128, 1], F32, name="vmid")
            vout = cpool.tile([128, 1], F32, name="vout")
            vsh = cpool.tile([128, 1], F32, name="vsh")
            cvin = cpool.tile([128, 1], F32, name="cvin")
            cvmid = cpool.tile([128, 1], F32, name="cvmid")
            obias = cpool.tile([128, 1], F32, name="obias")
            for smp in (0, 1):
                for q, dest in enumerate([vin, vmid, vout, vsh]):
                    gdma(
                        dest[64 * smp:64 * smp + 64, 0:1],
                        ksb[smp:smp + 1, 64 * q:64 * q + 64],
                    )
            nc.vector.tensor_scalar_add(cvin[:], vin[:], PI_2)
            nc.vector.tensor_scalar_add(cvmid[:], vmid[:], PI_2)
            nc.vector.tensor_add(obias[:], vout[:], vsh[:])

            # ---- sharded out/short hypernet + single AllToAll ------------
            wos_tiles = []
            for q in range(4):
                wt = cpool.tile([128, KOS], BF16, name=f"wos{q}",
                                tag=f"wos{q}")
                nc.sync.dma_start(wt[:], wosT_d[128 * q:128 * (q + 1), :])
                wos_tiles.append(wt)
            cc_in = dram_pool.tile([B, KOS], BF16, name="cc_in")
            cc_out = dram_pool.tile([B, KOS], BF16, name="cc_out")
            ks_os = cpool.tile([B, KOS], BF16, name="ks_os")
            for n0 in range(0, KOS, 512):
                ps = psC.tile([B, 512], F32, name="osps", tag="psC")
                for q in range(4):
                    nc.tensor.matmul(ps[:], lat_tiles[q][:],
                                     wos_tiles[q][:, n0:n0 + 512],
                                     start=(q == 0), stop=False)
                nc.tensor.matmul(ps[:], ones16[:], bkos[:, n0:n0 + 512],
                                 start=False, stop=True)
                nc.vector.tensor_copy(ks_os[:, n0:n0 + 512], ps[:])
                gdma(cc_in[:, n0:n0 + 512],
                                    ks_os[:, n0:n0 + 512])
            nc.gpsimd.collective_compute(
                "AllToAll",
                mybir.AluOpType.bypass,
                replica_groups=[list(range(NCORES))],
                ins=[cc_in.opt()],
                outs=[cc_out.opt()],
            )

            # ---- replicated-hypernet piece machinery ---------------------
            # kim piece p = kimT cols [1024p, 1024(p+1)): 4 SBUF band tiles.
            # k_in = pieces 0..3 (scalar queue), k_mid = 4..11 (sync queue).
            kim_pieces, bkim_pieces = {}, {}

            def load_kim_piece(p, queue):
                tiles = []
                for q in range(4):
                    kt = kim_pool.tile([128, 1024], BF16, name="kimp",
                                       tag=f"kimp{q}")
                    queue.dma_start(
                        kt[:], kimT_d[128 * q:128 * (q + 1),
                                      1024 * p:1024 * (p + 1)])
                    tiles.append(kt)
                kim_pieces[p] = tiles
                bt = bk_pool.tile([1, 1024], BF16, name="bkimp", tag="bkimp")
                gdma(
                    bt[:], bkim_d[0:1, 1024 * p:1024 * (p + 1)])
                bkim_pieces[p] = bt

            def hyper_chunk(n, dest_sb, dest_col):
                """512-col own-sample hypernet chunk n -> bf16 SBUF dest."""
                p, j = n // 2, n % 2
                kts, bt = kim_pieces[p], bkim_pieces[p]
                ps = psC.tile([2, 512], F32, name="kinps", tag="psC")
                for q in range(4):
                    nc.tensor.matmul(
                        ps[:], lat2_tiles[q][:],
                        kts[q][:, 512 * j:512 * (j + 1)],
                        start=(q == 0), stop=False)
                nc.tensor.matmul(ps[:], ones2[:],
                                 bt[:, 512 * j:512 * (j + 1)],
                                 start=False, stop=True)
                nc.vector.tensor_copy(dest_sb[:, dest_col:dest_col + 512],
                                      ps[:])
                if j == 1:
                    del kim_pieces[p], bkim_pieces[p]

            # ---- k_in hypernet (prologue) --------------------------------
            ks_in = cpool.tile([2, 4096], BF16, name="ks_in")
            ksd_mid = dram_pool.tile([2, 8192], BF16, name="ksd_mid")
            for p in range(4):
                load_kim_piece(p, nc.scalar)
                hyper_chunk(2 * p, ks_in, 1024 * p)
                hyper_chunk(2 * p + 1, ks_in, 1024 * p + 512)

            # ---- weight tiles --------------------------------------------
            W_in = w_pool.tile([128, 128], BF16, name="W_in")
            W_mid_c = w_pool.tile([128, 128], BF16, name="W_mid_c")
            W_mid_s = w_pool.tile([128, 128], BF16, name="W_mid_s")
            W_out_c = w_pool.tile([128, 128], BF16, name="W_out_c")
            W_out_s = w_pool.tile([128, 128], BF16, name="W_out_s")
            W_short = w_pool.tile([128, 128], BF16, name="W_short")
            for Wt in (W_in, W_mid_c, W_mid_s, W_out_c, W_out_s, W_short):
                nc.gpsimd.memset(Wt[0:64, 64:128], 0.0)
                nc.gpsimd.memset(Wt[64:128, 0:64], 0.0)
            for smp in (0, 1):
                dg = np.s_[64 * smp:64 * smp + 64, 64 * smp:64 * smp + 64]
                gdma(W_in[dg], ks_in[smp:smp + 1, :])

            # ---- main loop -----------------------------------------------
            xts, x2ts, w1s_, w2s_ = {}, {}, {}, {}
            ps_ins, ps_mids = {}, {}
            prev_act = None
            for t in range(NT + D + 3):
                u = t - 1 - D          # mid/w2 tile
                v = t - 2 - D          # out tile

                if 0 <= u < NT:
                    w1cs = w1s_.pop(u)
                    ps_mid = psB.tile([128, S], F32, name="ps_mid",
                                      tag="psB")
                    for h in range(2):
                        sl = np.s_[:, 512 * h:512 * (h + 1)]
                        nc.tensor.matmul(ps_mid[sl], W_mid_c[:],
                                         w1cs[:, 0:S][sl], start=True,
                                         stop=False)
                        nc.tensor.matmul(ps_mid[sl], W_mid_s[:],
                                         w1cs[:, S:2 * S][sl], start=False,
                                         stop=True)
                    ps_mids[u] = ps_mid

                if v == 0:
                    # out/short weight assembly, emitted late so the A2A
                    # wait cannot head-block earlier gpsimd-queue work.
                    for smp in (0, 1):
                        dg = np.s_[64 * smp:64 * smp + 64,
                                   64 * smp:64 * smp + 64]
                        gdma(W_out_c[dg],
                                            cc_out[smp:8:2, 0:1024])
                        gdma(W_out_s[dg],
                                            cc_out[8 + smp:16:2, 0:1024])
                        gdma(W_short[dg],
                                            cc_out[smp:16:2, 1024:1536])

                if 0 <= v < NT:
                    w2cs = w2s_.pop(v)
                    xt_v = x2ts.pop(v)
                    ot = out_pool.tile([128, S], BF16, name="ot", tag="ot")
                    for h in range(2):
                        sl = np.s_[:, 512 * h:512 * (h + 1)]
                        ps_out = psC.tile([128, 512], F32, name="ps_out",
                                          tag="psC")
                        nc.tensor.matmul(ps_out[:], W_out_c[:],
                                         w2cs[:, 0:S][sl], start=True,
                                         stop=False)
                        nc.tensor.matmul(ps_out[:], W_out_s[:],
                                         w2cs[:, S:2 * S][sl], start=False,
                                         stop=False)
                        nc.tensor.matmul(ps_out[:], W_short[:], xt_v[sl],
                                         start=False, stop=True)
                        nc.vector.tensor_scalar_add(ot[sl], ps_out[:],
                                                    obias[:, 0:1])
                    nc.sync.dma_start(y_d[:, S * v:S * (v + 1)], ot[:])

                if t < NT:
                    xt = x_pool.tile([128, S], BF16, name="xt", tag="xt")
                    nc.sync.dma_start(xt[:], x_d[:, S * t:S * (t + 1)])
                    xts[t] = xt
                    # k_mid piece prefetch, 4 blocks ahead of its use
                    if t % 2 == 0 and 4 + t // 2 <= 11:
                        load_kim_piece(4 + t // 2, nc.sync)
                    pool = psB if (t % 2 == 1 and t < D + 1) else psA
                    ps_in = pool.tile([128, S], F32, name="ps_in",
                                      tag="psB" if pool is psB else "psA")
                    for h in range(2):
                        sl = np.s_[:, 512 * h:512 * (h + 1)]
                        nc.tensor.matmul(ps_in[sl], W_in[:], xt[sl],
                                         start=True, stop=True)
                    ps_ins[t] = ps_in

                if 0 <= t - 1 < NT:
                    w = t - 1
                    w1cs = w1_pool.tile([128, 2 * S], BF16, name="w1",
                                        tag="w1")
                    ps_in_w = ps_ins.pop(w)
                    a = nc.scalar.activation(w1cs[:, 0:S], ps_in_w[:], SIN,
                                             bias=cvin[:, 0:1])
                    prev_act = _chain(prev_act, a)
                    a = nc.scalar.activation(w1cs[:, S:2 * S], ps_in_w[:],
                                             SIN, bias=vin[:, 0:1])
                    prev_act = _chain(prev_act, a)
                    w1s_[w] = w1cs

                if 0 <= u < NT:
                    w2cs = w2_pool.tile([128, 2 * S], BF16, name="w2",
                                        tag="w2")
                    ps_mid_u = ps_mids.pop(u)
                    a = nc.scalar.activation(w2cs[:, 0:S], ps_mid_u[:], SIN,
                                             bias=cvmid[:, 0:1])
                    prev_act = _chain(prev_act, a)
                    a = nc.scalar.activation(w2cs[:, S:2 * S], ps_mid_u[:],
                                             SIN, bias=vmid[:, 0:1])
                    prev_act = _chain(prev_act, a)
                    w2s_[u] = w2cs

                # k_mid hypernet drip: one 512-col chunk per block
                n = t - 4
                if 0 <= n < NKMID:
                    stg = stg_pool.tile([2, 512], BF16, name="stg",
                                        tag="stg")
                    hyper_chunk(8 + n, stg, 0)
                    gdma(ksd_mid[:, 512 * n:512 * (n + 1)],
                                        stg[:])
                    if n == NKMID - 1:
                        for smp in (0, 1):
                            dg = np.s_[64 * smp:64 * smp + 64,
                                       64 * smp:64 * smp + 64]
                            gdma(
                                W_mid_c[dg], ksd_mid[smp:smp + 1, 0:4096])
                            gdma(
                                W_mid_s[dg],
                                ksd_mid[smp:smp + 1, 4096:8192])

                # x re-fetch for the out stage (one block ahead)
                w = t - 1 - D
                if 0 <= w < NT:
                    xt2 = x2_pool.tile([128, S], BF16, name="xt2", tag="xt2")
                    nc.sync.dma_start(xt2[:], x_d[:, S * w:S * (w + 1)])
                    x2ts[w] = xt2

    nc.compile()
    return nc


_NC_CACHE = None


def _get_nc():
    global _NC_CACHE
    if _NC_CACHE is None:
        _NC_CACHE = _build_nc()
    return _NC_CACHE


def kernel(x, lat, Wk, bk, **run_kwargs):
    x = np.asarray(x, dtype=np.float32)
    lat = np.asarray(lat, dtype=np.float32)
    Wk = np.asarray(Wk, dtype=np.float32)
    bk = np.asarray(bk, dtype=np.float32)

    idx_kim, idx_os, scale = _build_indices()
    Wk_s = Wk * scale[:, None]
    bk_s = bk * scale
    latT_b = np.ascontiguousarray(lat.T.astype(NP_BF16))
    x_b = x.reshape(B, FIN * SP).astype(NP_BF16)
    kimT_b = np.ascontiguousarray(Wk_s[idx_kim].T.astype(NP_BF16))
    bkim_b = np.ascontiguousarray(bk_s[idx_kim].reshape(1, KIM)
                                  .astype(NP_BF16))
    bT_b = np.ascontiguousarray(Wk[24576:24832].T.astype(NP_BF16))
    bk2_b = np.ascontiguousarray(bk[24576:24832].reshape(1, 256)
                                 .astype(NP_BF16))

    in_maps = []
    for c in range(NCORES):
        in_maps.append({
            "x": np.ascontiguousarray(
                x_b[2 * c:2 * c + 2].reshape(128, SP)),
            "latT": latT_b,
            "lat2": np.ascontiguousarray(latT_b[:, 2 * c:2 * c + 2]),
            "kimT": kimT_b,
            "bkim": bkim_b,
            "wosT": np.ascontiguousarray(Wk_s[idx_os[c]].T.astype(NP_BF16)),
            "bkos": np.ascontiguousarray(bk_s[idx_os[c]].reshape(1, KOS)
                                         .astype(NP_BF16)),
            "bT": bT_b,
            "bk2": bk2_b,
            "ones2": np.ones((1, 2), NP_BF16),
            "ones16": np.ones((1, B), NP_BF16),
            "zeros": np.zeros((16, 16), NP_BF16),
        })

    nc = _get_nc()
    res = run_bass_kernel_spmd(nc, in_maps, core_ids=list(range(NCORES)),
                               **run_kwargs)
    y = np.empty((B, FOUT, HH, WW), np.float32)
    for c in range(NCORES):
        y[2 * c:2 * c + 2] = (res.results[c]["y"].astype(np.float32)
                              .reshape(2, FOUT, HH, WW))
    if run_kwargs:
        kernel.last_results = res
    return y
